# revision 13
# baseline (speedup 1.0000x reference)
"""Trainium2 Bass kernel for nn_DirModel (quaternion Dirac GNN message passing).

Strategy (8 NeuronCores, B=2 samples; core c owns sample s=c//4, slice r=c%4):
  - Di/DiA host-transposed/sliced, fp8, resident in SBUF (8+8 MB/core).
  - Per-core STATE = only the OWNED slice of f (512 rows) and v (256 rows),
    kept resident in PSUM in the big-matmul output layout; the residual add
    is folded into the matmul accumulation (start=False).
  - After each big matmul, the core applies elu to its PSUM slice, computes
    per-channel partial (sum, sumsq) via accum_out, and AllGathers
    [elu'd slice (fp8) | stats (8 bytes)] in ONE collective. Other cores
    never re-apply elu; BN stats come from summing the gathered partials.
  - BatchNorm is folded into the per-node linear: W' = W * (rstd*gamma*sel)
    (sel zeroes the non-owned sample, baked host-side into gamma/beta),
    bias' = b - (mean*scale - beta) @ W via a tiny matmul. The linear's
    elementwise tail (bias+elu) runs fused over 4 node-chunks at a time.
  - Big matmuls: activations stationary ([128,16] fp8 slices), resident
    fp8 operators streamed, 4-way PE column tiling, PSUM accumulation.
  - The initial v AllGather doubles as collective warm-up, overlapped with
    the resident operator DMA streaming.
"""

import numpy as np
import ml_dtypes

import concourse.bass as bass
import concourse.mybir as mybir
import concourse.tile as tile
from concourse import bacc
from concourse.bass_utils import run_bass_kernel_spmd

B, N, F, C = 2, 1024, 2048, 64
NB = 5
EPS = 1e-5
NCORES = 8
NSL = N // 4   # 256 nodes per slice
FSL = F // 4   # 512 faces per slice

F32 = mybir.dt.float32
BF16 = mybir.dt.bfloat16
FP8 = mybir.dt.float8e4
NP_BF16 = ml_dtypes.bfloat16
NP_FP8 = ml_dtypes.float8_e4m3
AF = mybir.ActivationFunctionType
ALU = mybir.AluOpType
RG = [list(range(NCORES))]


def _build():
    nc = bacc.Bacc(
        "TRN2",
        target_bir_lowering=False,
        debug=False,
        enable_asserts=False,
        num_devices=NCORES,
    )

    # ---------------- DRAM I/O ----------------
    dit_d = nc.dram_tensor("dit", [128, 32, 2048], FP8, kind="ExternalInput")
    diat_d = nc.dram_tensor("diat", [128, 64, 1024], FP8, kind="ExternalInput")
    inpTm_d = nc.dram_tensor("inpTm", [3, NSL], BF16, kind="ExternalInput")
    winp_d = nc.dram_tensor("winp", [3, 128], BF16, kind="ExternalInput")
    binp_d = nc.dram_tensor("binp", [1, 128], BF16, kind="ExternalInput")
    w0_d = nc.dram_tensor("w0", [128, NB, C], BF16, kind="ExternalInput")
    w1_d = nc.dram_tensor("w1", [128, NB, C], BF16, kind="ExternalInput")
    b0_d = nc.dram_tensor("b0", [1, NB, C], BF16, kind="ExternalInput")
    b1_d = nc.dram_tensor("b1", [1, NB, C], BF16, kind="ExternalInput")
    g0_d = nc.dram_tensor("g0", [128, NB], F32, kind="ExternalInput")
    be0_d = nc.dram_tensor("be0", [128, NB], F32, kind="ExternalInput")
    g1_d = nc.dram_tensor("g1", [128, NB], F32, kind="ExternalInput")
    be1_d = nc.dram_tensor("be1", [128, NB], F32, kind="ExternalInput")
    bn2g_d = nc.dram_tensor("bn2g", [128, 1], F32, kind="ExternalInput")
    bn2b_d = nc.dram_tensor("bn2b", [128, 1], F32, kind="ExternalInput")
    w2_d = nc.dram_tensor("w2", [128, C], BF16, kind="ExternalInput")
    b2_d = nc.dram_tensor("b2", [1, C], BF16, kind="ExternalInput")
    maskc_d = nc.dram_tensor("maskc", [128, 8, B], BF16, kind="ExternalInput")
    maskrow_d = nc.dram_tensor("maskrow", [B, N], BF16, kind="ExternalInput")
    wfc_d = nc.dram_tensor("wfc", [C, 10], BF16, kind="ExternalInput")
    bfc_d = nc.dram_tensor("bfc", [B, 10], F32, kind="ExternalInput")
    out_d = nc.dram_tensor("out", [B, 10], F32, kind="ExternalOutput")

    with tile.TileContext(nc) as tc:
        with (
            tc.tile_pool(name="res", bufs=1) as res,
            tc.tile_pool(name="sb", bufs=2) as sb,
            tc.tile_pool(name="sc", bufs=2) as sc,
            tc.tile_pool(name="st", bufs=4) as st,
            tc.tile_pool(name="pacc", bufs=1, space="PSUM") as pacc,
            tc.tile_pool(name="px", bufs=2, space="PSUM") as px,
            tc.tile_pool(name="pm", bufs=1, space="PSUM") as pm,
            tc.tile_pool(name="dram", bufs=2, space="DRAM") as dram,
        ):
            # ------------- small resident loads (sync queue, first) -------
            def load(name, shape, dtype, src):
                t = res.tile(shape, dtype, name=name)
                nc.sync.dma_start(t[:], src.ap())
                return t

            inpTm_sb = load("inpTm_sb", [3, NSL], BF16, inpTm_d)
            winp_sb = load("winp_sb", [3, 128], BF16, winp_d)
            binp_sb = load("binp_sb", [1, 128], BF16, binp_d)
            w0_sb = load("w0_sb", [128, NB, C], BF16, w0_d)
            w1_sb = load("w1_sb", [128, NB, C], BF16, w1_d)
            b0_sb = load("b0_sb", [1, NB, C], BF16, b0_d)
            b1_sb = load("b1_sb", [1, NB, C], BF16, b1_d)
            g0_sb = load("g0_sb", [128, NB], F32, g0_d)
            be0_sb = load("be0_sb", [128, NB], F32, be0_d)
            g1_sb = load("g1_sb", [128, NB], F32, g1_d)
            be1_sb = load("be1_sb", [128, NB], F32, be1_d)
            bn2g_sb = load("bn2g_sb", [128, 1], F32, bn2g_d)
            bn2b_sb = load("bn2b_sb", [128, 1], F32, bn2b_d)
            w2_sb = load("w2_sb", [128, C], BF16, w2_d)
            b2_sb = load("b2_sb", [1, C], BF16, b2_d)
            maskc_sb = load("maskc_sb", [128, 8, B], BF16, maskc_d)
            maskrow_sb = load("maskrow_sb", [B, N], BF16, maskrow_d)
            wfc_sb = load("wfc_sb", [C, 10], BF16, wfc_d)
            bfc_sb = load("bfc_sb", [B, 10], F32, bfc_d)

            # big resident operators, in consumption order, split across two
            # engine queues so issue cost doesn't serialize either stream.
            dit_v = dit_d.ap().rearrange("p (a k) n -> p a k n", a=16)
            dit_cs = []
            for a in range(16):
                t = res.tile([128, 2, 2048], FP8, name=f"dit{a}")
                nc.sync.dma_start(t[:], dit_v[:, a])
                dit_cs.append(t)
            diat_v = diat_d.ap().rearrange("p (a k) n -> p a k n", a=16)
            diat_cs = []
            for a in range(16):
                t = res.tile([128, 4, 1024], FP8, name=f"diat{a}")
                nc.sync.dma_start(t[:], diat_v[:, a])
                diat_cs.append(t)

            ones1 = res.tile([1, 128], BF16)
            nc.vector.memset(ones1[:], 1.0)
            onesrow = res.tile([1, NSL], BF16)
            nc.vector.memset(onesrow[:], 1.0)

            # ------------- persistent PSUM state (slice layout) -----------
            # rows 32j+c (j=0..3, c=0..15) hold channel 16j+c of the OWNED
            # sample; cols are the owned rows of f / v (local order).
            psfB = pacc.tile([128, FSL], F32, name="psfB")
            psvB = pacc.tile([128, NSL], F32, name="psvB")

            # initial v = inputs @ W_in + b_in (padded channel layout)
            nc.tensor.matmul(psvB[:], winp_sb[:], inpTm_sb[:],
                             start=True, stop=False)
            nc.tensor.matmul(psvB[:], binp_sb[:], onesrow[:],
                             start=False, stop=True)

            def exchange(state_ps, R, nm):
                """elu(state slice) + stats -> AllGather -> (eB, stat128).

                state_ps: [128, R] PSUM (padded 32j+c rows).
                Returns eB [128, 4*R] fp8 (partition 64s+c) and
                stat128 [128, 64] fp8 (= [128,16] f32: 8 groups x (sum, sq)).
                """
                stg = sc.tile([128, R + 8], FP8, tag=f"stg{nm[0]}",
                              name=f"stg{nm}")
                e2 = sc.tile([128, R], BF16, tag=f"e2x{nm[0]}",
                             name=f"e2x{nm}")
                sqd = sc.tile([128, R], BF16, tag=f"sqd{nm[0]}",
                              name=f"sqd{nm}")
                nc.scalar.activation(e2[:], state_ps[:], AF.Exp)
                nc.vector.tensor_scalar(e2[:], e2[:], -1.0, 0.0, ALU.add, ALU.min)
                nc.vector.scalar_tensor_tensor(
                    stg[:, 0:R], state_ps[:], 0.0, e2[:], ALU.max, ALU.add,
                    accum_out=stg[:, R:R + 4].bitcast(F32),
                )
                nc.scalar.activation(sqd[:], stg[:, 0:R], AF.Square,
                                     accum_out=stg[:, R + 4:R + 8].bitcast(F32))
                # compact 4x16 padded rows -> 64 rows, stage to DRAM
                agin = dram.tile([C, R + 8], FP8, tag=f"agin{nm[0]}",
                                 name=f"agin{nm}")
                cengs = [nc.sync, nc.scalar, nc.sync, nc.scalar]
                for j in range(4):
                    cengs[j].dma_start(
                        agin[16 * j:16 * (j + 1), :],
                        stg[32 * j:32 * j + 16, :],
                    )
                agout = dram.tile([NCORES * C, R + 8], FP8, tag=f"agout{nm[0]}",
                                  name=f"agout{nm}", addr_space="Shared")
                nc.gpsimd.collective_compute(
                    "AllGather", ALU.bypass, replica_groups=RG,
                    ins=[agin.opt()], outs=[agout.opt()],
                )
                stat128 = st.tile([128, 64], FP8, tag="stat128",
                                  name=f"s128{nm}")
                statsrc = agout[:, R:R + 8].rearrange("(g c) x -> c g x", g=8)
                s3 = stat128[:].rearrange("p (g x) -> p g x", g=8)
                nc.gpsimd.dma_start(s3[0:C], statsrc)
                nc.gpsimd.dma_start(s3[C:128], statsrc)
                # values fetched in parallel on the (idle) sync/scalar queues
                eB = sb.tile([128, 4 * R], FP8, tag=f"eB{nm[0]}",
                             name=f"eB{nm}")
                eBv = eB[:].rearrange("p (r n) -> p r n", r=4)
                for s in range(B):
                    (nc.sync if s == 0 else nc.scalar).dma_start(
                        eBv[C * s:C * (s + 1), :, :],
                        agout[256 * s:256 * (s + 1), 0:R].rearrange(
                            "(r c) n -> c r n", r=4),
                    )
                return eB, stat128

            def bn_fold(stat128, g_ap, be_ap, w_ap, b_ap, T, nm,
                        head=False):
                """stats -> (W' [128,C] fp8, bias_row [1,C] bf16).

                head=True: gamma/beta are nonzero on BOTH sample halves, so
                the K=128 u@W matmul double-counts; halve it."""
                sview = stat128[:].bitcast(F32).rearrange(
                    "p (g x) -> p x g", g=8)
                sv = st.tile([128, 1], F32, tag="bns", name=f"sv{nm}")
                nc.vector.tensor_reduce(sv[:], sview[:, 0:1, :],
                                        mybir.AxisListType.X, ALU.add)
                qv = st.tile([128, 1], F32, tag="bns2", name=f"qv{nm}")
                nc.vector.tensor_reduce(qv[:], sview[:, 1:2, :],
                                        mybir.AxisListType.X, ALU.add)
                mean = st.tile([128, 1], F32, tag="bns3", name=f"mn{nm}")
                nc.vector.tensor_scalar_mul(mean[:], sv[:], 1.0 / T)
                m2 = st.tile([128, 1], F32, tag="bns4", name=f"m2{nm}")
                nc.vector.tensor_mul(m2[:], mean[:], mean[:])
                varp = st.tile([128, 1], F32, tag="bns5", name=f"vp{nm}")
                nc.vector.scalar_tensor_tensor(
                    varp[:], qv[:], 1.0 / T, m2[:], ALU.mult, ALU.subtract)
                nc.vector.tensor_scalar_add(varp[:], varp[:], EPS)
                # rsqrt: bit-trick + 2 Newton steps (ACT tables stay on EXP)
                iv = st.tile([128, 1], mybir.dt.int32, tag="bns6",
                             name=f"iv{nm}")
                nc.vector.tensor_scalar(
                    iv[:], varp[:].bitcast(mybir.dt.int32), 1, None,
                    ALU.arith_shift_right)
                nc.vector.tensor_scalar(
                    iv[:], iv[:], -1, 0x5F3759DF, ALU.mult, ALU.add)
                rstd = st.tile([128, 1], F32, tag="bns7", name=f"rstd{nm}")
                nc.vector.tensor_copy(rstd[:], iv[:].bitcast(F32))
                nt = st.tile([128, 1], F32, tag="bns8", name=f"nt{nm}")
                for _ in range(1):
                    nc.vector.tensor_mul(nt[:], rstd[:], rstd[:])
                    nc.vector.tensor_mul(nt[:], nt[:], varp[:])
                    nc.vector.tensor_scalar(
                        nt[:], nt[:], -0.5, 1.5, ALU.mult, ALU.add)
                    nc.vector.tensor_mul(rstd[:], rstd[:], nt[:])
                scale = st.tile([128, 1], F32, tag="bns9", name=f"sc{nm}")
                nc.vector.tensor_mul(scale[:], rstd[:], g_ap)
                wp = sc.tile([128, C], FP8, tag="wp", name=f"wp{nm}")
                nc.vector.tensor_scalar(wp[:], w_ap, scale[:].opt(), None,
                                        ALU.mult)
                u = st.tile([128, 1], BF16, tag="bnsu", name=f"u{nm}")
                nc.vector.scalar_tensor_tensor(
                    u[:], mean[:], scale[:].opt(), be_ap, ALU.mult,
                    ALU.subtract)
                psu = pm.tile([1, C], F32, tag="psu", name=f"psu{nm}")
                nc.tensor.matmul(psu[:], u[:], w_ap, start=True, stop=True)
                brow = st.tile([1, C], BF16, tag="brow", name=f"br{nm}")
                nc.vector.scalar_tensor_tensor(
                    brow[:], psu[:], -0.5 if head else -1.0, b_ap,
                    ALU.mult, ALU.add)
                brow4 = st.tile([1, 4 * C], BF16, tag="brow4",
                                name=f"br4{nm}")
                for t in range(4):
                    nc.vector.tensor_copy(brow4[:, C * t:C * (t + 1)],
                                          brow[:])
                return wp, brow4

            def rows_group(eB, base, wp, brow, nm, k64s=None):  # brow: [1, 4*C]
                """4 node-chunks: x4 = elu(eB_chunks @ W' + bias) [128,4*C] fp8.

                k64s: if set, contract only K=64 partitions [k64s, k64s+64)
                (head per-sample path)."""
                ps4 = px.tile([128, 4 * C], F32, tag="ps4", name=f"ps4{nm}")
                for t in range(4):
                    cs = slice(C * t, C * (t + 1))
                    if k64s is None:
                        nc.tensor.matmul(
                            ps4[:, cs],
                            eB[:, 128 * (base + t):128 * (base + t + 1)],
                            wp[:], start=True, stop=False)
                    else:
                        nc.tensor.matmul(
                            ps4[:, cs],
                            eB[k64s:k64s + C,
                               128 * (base + t):128 * (base + t + 1)],
                            wp[k64s:k64s + C, :], start=True, stop=False)
                nc.tensor.matmul(ps4[:], ones1[:], brow[:],
                                 start=False, stop=True)
                e2 = sc.tile([128, 4 * C], BF16, tag="e2r", name=f"e2r{nm}")
                nc.scalar.activation(e2[:], ps4[:], AF.Exp)
                nc.vector.tensor_scalar(e2[:], e2[:], -1.0, 0.0, ALU.add,
                                        ALU.min)
                x4 = sc.tile([128, 4 * C],
                             FP8 if k64s is None else BF16,
                             tag="x4" if k64s is None else "x4h",
                             name=f"x4{nm}")
                nc.vector.scalar_tensor_tensor(
                    x4[:], ps4[:], 0.0, e2[:], ALU.max, ALU.add)
                return x4

            # =================== main loop ===================
            # initial v exchange (also collective warm-up)
            e_vB, stat_v = exchange(psvB, NSL, "vi")

            for i in range(NB):
                # -------- v side: x = elu(BN-linear(e_v)); Di @ x --------
                w0p, b0row = bn_fold(
                    stat_v, g0_sb[:, i:i + 1].opt(), be0_sb[:, i:i + 1].opt(),
                    w0_sb[:, i, :], b0_sb[:, i, :], float(B * N), f"v{i}")
                for g in range(2):
                    x4 = rows_group(e_vB, 4 * g, w0p, b0row, f"x{i}g{g}")
                    for t in range(4):
                        n8 = 4 * g + t
                        for jj in range(4):
                            kk = 4 * n8 + jj
                            for j in range(4):
                                nc.tensor.matmul(
                                    psfB[32 * j:32 * j + 16, :],
                                    x4[:, C * t + 16 * jj:C * t + 16 * (jj + 1)],
                                    dit_cs[kk // 2][:, kk % 2,
                                                    512 * j:512 * (j + 1)],
                                    start=(i == 0 and kk == 0),
                                    stop=(kk == 31),
                                    tile_position=(0, 32 * j),
                                )
                # -------- exchange f --------
                e_fB, stat_f = exchange(psfB, FSL, f"f{i}")

                # -------- f side: y = elu(BN-linear(e_f)); DiA @ y --------
                w1p, b1row = bn_fold(
                    stat_f, g1_sb[:, i:i + 1].opt(), be1_sb[:, i:i + 1].opt(),
                    w1_sb[:, i, :], b1_sb[:, i, :], float(B * F), f"f{i}")
                for g in range(4):
                    y4 = rows_group(e_fB, 4 * g, w1p, b1row, f"y{i}g{g}")
                    for t in range(4):
                        pc = 4 * g + t
                        for jj in range(4):
                            kk = 4 * pc + jj
                            for j in range(4):
                                nc.tensor.matmul(
                                    psvB[32 * j:32 * j + 16, :],
                                    y4[:, C * t + 16 * jj:C * t + 16 * (jj + 1)],
                                    diat_cs[kk // 4][:, kk % 4,
                                                     256 * j:256 * (j + 1)],
                                    start=False,
                                    stop=(kk == 63),
                                    tile_position=(0, 32 * j),
                                )
                # -------- exchange v --------
                e_vB, stat_v = exchange(psvB, NSL, f"v{i}")

            # =================== head ===================
            w2p, b2row = bn_fold(
                stat_v, bn2g_sb[:].opt(), bn2b_sb[:].opt(),
                w2_sb[:], b2_sb[:], float(B * N), "h", head=True)
            pooled = sb.tile([C, B], BF16, tag="pooled")
            for s in range(B):
                pp = pm.tile([C, 1], F32, tag="pp", name=f"pp{s}")
                for g in range(2):
                    r4 = rows_group(e_vB, 4 * g, w2p, b2row, f"h{s}g{g}",
                                    k64s=C * s)
                    for t in range(4):
                        nc.tensor.matmul(
                            pp[:], r4[:, C * t:C * (t + 1)],
                            maskc_sb[:, 4 * g + t, s:s + 1],
                            start=(g == 0 and t == 0),
                            stop=(g == 1 and t == 3),
                        )
                nc.vector.tensor_copy(pooled[:, s:s + 1], pp[:])
            msum = st.tile([B, 1], F32, tag="hd", name="msum")
            nc.vector.tensor_reduce(
                msum[:], maskrow_sb[:], mybir.AxisListType.X, ALU.add)
            rec = st.tile([B, 1], F32, tag="hd", name="rec")
            nc.vector.reciprocal(rec[:], msum[:])
            pl = pm.tile([B, 10], F32, tag="pl", name="pl")
            nc.tensor.matmul(pl[:], pooled[:], wfc_sb[:], start=True, stop=True)
            lu = sb.tile([B, 10], F32, tag="hd2", name="lu")
            nc.vector.scalar_tensor_tensor(
                lu[:], pl[:], rec[:].opt(), bfc_sb[:], ALU.mult, ALU.add)
            rmax = st.tile([B, 1], F32, tag="hd", name="rmax")
            nc.vector.tensor_reduce(rmax[:], lu[:], mybir.AxisListType.X,
                                    ALU.max)
            t2 = sb.tile([B, 10], F32, tag="hd2", name="t2")
            nc.vector.tensor_scalar(t2[:], lu[:], rmax[:].opt(), None,
                                    ALU.subtract)
            et = sb.tile([B, 10], F32, tag="hd2", name="et")
            se = st.tile([B, 1], F32, tag="hd", name="se")
            nc.scalar.activation(et[:], t2[:], AF.Exp, accum_out=se[:])
            ls = st.tile([B, 1], F32, tag="hd", name="ls")
            nc.scalar.activation(ls[:], se[:], AF.Ln)
            outv = sb.tile([B, 10], F32, tag="hd2", name="outv")
            nc.vector.tensor_scalar(outv[:], t2[:], ls[:].opt(), None,
                                    ALU.subtract)
            nc.sync.dma_start(out_d.ap(), outv[:])

    nc.compile()
    return nc


_NC = None


def _get_nc():
    global _NC
    if _NC is None:
        _NC = _build()
    return _NC


def _pad128(a):
    """[*, 64] channel vector -> [*, 128] padded 32j+c layout (gaps zero)."""
    out = np.zeros(a.shape[:-1] + (128,), np.float32)
    for j in range(4):
        out[..., 32 * j:32 * j + 16] = a[..., 16 * j:16 * j + 16]
    return out


def _host_prep(inputs):
    """Build the 8 per-core input maps. Core c: sample s=c//4, slice r=c%4."""
    Di = np.ascontiguousarray(np.asarray(inputs["Di"]), np.float32)
    DiA = np.ascontiguousarray(np.asarray(inputs["DiA"]), np.float32)
    inp = np.asarray(inputs["inputs"], np.float32)
    mask = np.asarray(inputs["mask"], np.float32)[:, :, 0]   # [2, 1024]

    def dup(a):  # stack for both sample halves on K
        return np.concatenate([a, a], axis=0)

    base = {}
    base["w0"] = dup(np.ascontiguousarray(
        np.asarray(inputs["rn_W0"]).transpose(1, 0, 2))).astype(NP_BF16)
    base["w1"] = dup(np.ascontiguousarray(
        np.asarray(inputs["rn_W1"]).transpose(1, 0, 2))).astype(NP_BF16)
    base["b0"] = np.asarray(inputs["rn_b0"]).astype(NP_BF16)[None, :, :]
    base["b1"] = np.asarray(inputs["rn_b1"]).astype(NP_BF16)[None, :, :]
    base["bn2g"] = np.tile(
        np.asarray(inputs["bn2_g"]).astype(np.float32).reshape(C, 1), (2, 1))
    base["bn2b"] = np.tile(
        np.asarray(inputs["bn2_b"]).astype(np.float32).reshape(C, 1), (2, 1))
    base["w2"] = dup(np.asarray(inputs["W2"])).astype(NP_BF16)
    base["b2"] = np.asarray(inputs["b2"]).astype(NP_BF16).reshape(1, C)
    base["wfc"] = np.asarray(inputs["Wfc"]).astype(NP_BF16)
    base["bfc"] = np.broadcast_to(
        np.asarray(inputs["bfc"], np.float32), (B, 10)).copy()
    base["maskc"] = np.ascontiguousarray(
        mask.reshape(2, 8, 128).transpose(2, 1, 0)).astype(NP_BF16)
    base["maskrow"] = mask.astype(NP_BF16)
    # padded-channel W_in/b_in for the PSUM-layout v init
    base["winp"] = np.ascontiguousarray(
        _pad128(np.asarray(inputs["W_in"], np.float32))).astype(NP_BF16)
    base["binp"] = _pad128(
        np.asarray(inputs["b_in"], np.float32)).reshape(1, 128).astype(NP_BF16)

    g0 = np.tile(np.ascontiguousarray(
        np.asarray(inputs["rn_g0"]).T).astype(np.float32), (2, 1))
    be0 = np.tile(np.ascontiguousarray(
        np.asarray(inputs["rn_be0"]).T).astype(np.float32), (2, 1))
    g1 = np.tile(np.ascontiguousarray(
        np.asarray(inputs["rn_g1"]).T).astype(np.float32), (2, 1))
    be1 = np.tile(np.ascontiguousarray(
        np.asarray(inputs["rn_be1"]).T).astype(np.float32), (2, 1))

    in_maps = []
    for c in range(NCORES):
        s, r = c // 4, c % 4
        m = dict(base)
        Dr = Di[s].reshape(F, 4, N, 4)          # [p, j, n, jj]
        P4 = Dr[512 * r:512 * (r + 1)]          # [512, 4, 1024, 4]
        DiTg = P4.reshape(512, 4, 8, 128, 4).transpose(2, 4, 3, 1, 0) \
                 .reshape(4096, 2048)           # rows (n8,jj,n'), cols (j,p')
        m["dit"] = np.ascontiguousarray(
            DiTg.reshape(32, 128, 2048).transpose(1, 0, 2)).astype(NP_FP8)
        A = DiA[s].reshape(N, 4, F, 4)          # [n, j, p, jj]
        A4 = A[256 * r:256 * (r + 1)]           # [256, 4, 2048, 4]
        DiATg = A4.reshape(256, 4, 16, 128, 4).transpose(2, 4, 3, 1, 0) \
                  .reshape(8192, 1024)          # rows (pc,jj,p''), cols (j,n')
        m["diat"] = np.ascontiguousarray(
            DiATg.reshape(64, 128, 1024).transpose(1, 0, 2)).astype(NP_FP8)
        # per-core node slice of the inputs, transposed
        m["inpTm"] = np.ascontiguousarray(
            inp[s, NSL * r:NSL * (r + 1), :].T).astype(NP_BF16)
        # selector folded into gamma/beta: other sample's half zeroed
        msk = np.zeros((128, 1), np.float32)
        msk[64 * s:64 * (s + 1)] = 1.0
        m["g0"] = g0 * msk
        m["be0"] = be0 * msk
        m["g1"] = g1 * msk
        m["be1"] = be1 * msk
        in_maps.append(m)
    return in_maps


def _run(inputs, trace=False, **kw):
    nc = _get_nc()
    in_maps = _host_prep(inputs)
    res = run_bass_kernel_spmd(
        nc, in_maps, core_ids=list(range(NCORES)), trace=trace, **kw
    )
    out = np.asarray(res.results[0]["out"], np.float32).copy()
    return out, res


def kernel(**inputs):
    out, _ = _run(inputs, trace=False)
    return out
